# revision 1
# baseline (speedup 1.0000x reference)
"""AngularAttention distributed Bass kernel for 8 TRN2 NeuronCores.

Sharding: data-parallel over batch (2) x tensor-parallel over heads (16 -> 4
per core).  Core c handles batch c//4, heads 4*(c%4) .. 4*(c%4)+3.  Each core
computes its 4 heads' attention and a partial output projection
out_local @ Wo_local^T -> [T, 1024]; the host sums the 4 partials per batch
element (row-parallel Wo reduction done host-side) and adds bo.

Math per head (reference):
  Q = l2norm(x @ Wq_h^T), K = l2norm(x @ Wk_h^T), V = x @ Wv_h^T
  s = Q K^T  (|s| <= ~0.65 for this data, so clip(+-0.999) is a no-op)
  scores = 1 - arccos(s)/pi = 0.5 + arcsin(s)/pi
  W = scores^8 (eps guards are no-ops: scores in [0.29, 0.72])
  out = (W / (rowsum(W)+1e-6)) @ V
arcsin(s) = arctan(s / sqrt(1 - s^2)) -> Abs_reciprocal_sqrt + Arctan ACT
table functions; ^8 = three squarings.

Everything on-device is computed transposed (S^T layout: keys on partitions,
queries on free axis) so that attention weights feed the PE as the moving
operand with N=512 and fp32r runs at 1 cycle/row.
"""

import sys, os

for _p in ("/opt/trn_rl_repo",):
    if _p not in sys.path:
        sys.path.insert(0, _p)

import math
import numpy as np
from contextlib import ExitStack

import concourse.bass as bass
import concourse.bacc as bacc
import concourse.mybir as mybir
import concourse.tile as tile
from concourse.tile_rust import add_dep_helper

F32 = mybir.dt.float32
F32R = mybir.dt.float32r
BF16 = mybir.dt.bfloat16
AF = mybir.ActivationFunctionType

D = 1024          # model dim
DK = 64           # head dim
HL = 4            # local heads per core
CL = HL * DK      # 256 local projection cols
N_CORES = 8


def _r(ap):
    """View an AP as float32r for full-rate PE matmuls (N>=256)."""
    return ap.bitcast(F32R)


class _ActChain:
    """Forces a total order on scalar-engine (ACT) instructions so the
    scheduler cannot interleave activation-table sets; bacc then inserts a
    minimal number of ACT_TABLE_LOADs (one per phase transition)."""

    def __init__(self):
        self.prev = None

    def add(self, bass_inst):
        ins = bass_inst.ins
        if self.prev is not None:
            add_dep_helper(ins, self.prev, sync=False,
                           reason="act-table phase order")
        self.prev = ins
        return bass_inst



# ---------------------------------------------------------------------------
# Custom ACT tables: the gelu bucket slot is re-fit to evaluate
#   g8(s) = (0.5 + arcsin(clip(s, +-0.999))/pi)^8
# and derivative_gelu to 1/(-4 z + 1e-6)  (rowsum reciprocal via scale=-0.25),
# collapsing the per-element arccos/pow8 chain into a single ACT instruction.
# The patch contains refitted cubic-spline bucket rows for
# gelu_and_others_bkt.bin; bucket geometry (ctrl/profile) is unchanged.
_ACT_PATCH_B64 = "UEsDBC0AAAAIAAAAIQBxd08z//////////8IABQAaWR4cy5ucHkBABAAkAMAAAAAAAB8AQAAAAAAAJ3Q106UARAGUHrvvbPUfxfXgiIKolixglhQseEKi6AIZDHeGJ/Cx7H33nt5EE1sh8Qn8EvOxXyZi8lcGRoZHB5NTrqYdCmYiC+MJ4LeUNA33RVEQ8HkXOJCIjY7NpeYiC/2A7GZhbh+YSo2HzeHO3u6o5Fo6HLov5OTJMmkkEoa6WSQSRbZLC7mkkc+BRRSRDEllFJGORVUUkU1NdRSRz0NNBKiiWZaaKWNdgLCROjgqkOvcZ0b3OQWt7nDXe5xnwc85BGPecJTnvGcF7zkFa95w1ve8Z4PfOQTn/nCV77xnR/85Be/+fPvicmkkEoa6WSQSRbZ5JBLHvkUUEgRxZRQShnlVFBJFdXUUEsd9TTQSIgmmmmhlTbaCQgToYMlRFnKMpazgk5WsoouVtPNGtbSQy/r6GM9G+hnI5vYzBa2so0BtrODnexiN3sYZIi9DLOP/RzgICMc4jBHGOUoxzjOCU4yxilinGacCeJMcoYppjnLOWY4zyzzJPgLUEsDBC0AAAAIAAAAIQCTRxsZ//////////8IABQAcm93cy5ucHkBABAAABkAAAAAAAAODQAAAAAAAJ3XeViO2f8H8EKLVI+hMCGldZLStD3d9/mc5ylLIutMiBayJDMp2bJEG61IC6XSQmTJlhRJ2cLUWIqyzNj3bIVBg+f78V8fc12/57p+579zXtf9x33uc73v90kb6zVmvLeqyhKVFRazZi+cGWYhGFmwOQ4W1kYWc0LCFoXNmO8fEjZr9rf1ETN+Xzgb1xfOnRE6G+eWdi5O1kbSn6yNIoz+n0PLrP8CMWHXFmZoNRgkmUGgUMiOq3QYA78EiEbHU9khczMY+NUPvZr4L+4TxL8q49nuzn0hY8cE9AFVHX3rOblYfXAVW77iBxg/eDi6P/Heplbip5JZrGRYF9jabo+eRzwoQSJO2jGc1dc8ZYsXD0C/Q1xu1iqE+Fix/PR6tjhQB11yoqPfyb0g/DmzB3u9qZRNftfOFApb4hmHdwqxR9vF1XNz2fALt9HHET9bnijMf3RNLDqSwOZfO4seQjyi5yxhSVmZeDYslBXu248eR/w1E4R127PEuJkT2ND2bPTNxO3e9hL8WJQ46Yoj6x2ciL6TeJfSNpd28yli8LT+LDluMXo58Z5TTrpUbLYX1+7VYOnx3ujniE9ZnunSPFBP/LvtqTh3pRy9mfiJ+mCXppltgq+iVnTqZoX+hPiOPs4u3ivPCLmau8ScZ/roH4iHlL6RTk9OFl7djxbTrD6ICoVGdUfXtYuSeh4eJKwPcRAlG4+j9ybu6P/ZWS3ntEvajZfCZq116BbEr56e7HykycMlWCNNKK7zRHcmrt+wyWm9tFQ6uMFUSLypg+5O3Np4k6Nvyz/OKzbmuLj2LRMUisnEXY7rOdi8uuAUtPCq9N8WN/RA4lfUTw7ZGxzg0Bbs6WxSPM9FoVhK/OG4G+Yfcnx2nZNNsvb1vWarUEQRVx381rz7GPUevhvzMu5/de2jUKz5j3ecKxQJSnydEt+oxDOUeJYSz1XiBUq8SInvUuIlSvygEi9T4keVeJUSP0k8vTlOPHtgD9s3g0Pk0Mhv+co6+lCDFHH41cPMsMAd0peu/ZavxD+/LxCHLD7Ppq+dBpYtad+eh46ee/G4WHjoEfM1XAZbHYq/5SfxiuRb4ugFnWFR4DqIP1T9LT+JO5V/Fft69II3eflQpbj5LT95R9dY1JM1HBoEX3KOQNPtdnRb4gcDrRjoDoMBzZeg1ag7x/wkXpQ9jFXV+YPx3Rfw1NIcPYS4XWwAqw9ZDWcdNPiG4XL0OOI3folmJ+o2wpIdBnxGkD/6ZuKfL2cwrdU7IGu0HV82Mxp9J3HQ28vs3U7A1brRfM/HTPRy4pfjzjKT3Bvw6dM87hNxGP0cfb/i+2y54yfQbI3j28Y0oTcTD1LrBL+16PK5r3L5KWMVGeYncXdbffjKTXmQVwU3WWiA/oH41whneOQ+mucFtPDrKePRNWQdvSU5DI7lxPPWTHOZfmQBem/idr8UwD2VSl54YpHMuKmLHPOTeMLd6yBZ0lmWkV4ls+46H92ZeMNIPT4nSS67JR8gL7vbhu5OvFvOeP72xSbZmi1Z8tg9W1wxP4m/PLCF6zhoyHUWjnCN6rPVDfOT+FP/TrLMgWXyyqlP3B696Twc85O4ybOZsl/7qVaHmye5uk69OEyhiCEe6zqLzBWKJCWepsSzlfg2Jb5HiZcq8UolfprMVVQixbhPhSw22REqtoaDyn9GpBiJvh39wKfQ//iS4N1y/xd/uf21qnXYE8e0EZgfpD9dvfKT/G+P2672nY8OfXw+Gb9PNfHBk3bLvMfel/vpWrs9l8wfis+T//+n5JmyFW6j5Pr2Tq7lmwbi999AXM9GkJ2aoyF3P1wrN7ibgudnP/FtCf1l6g9PygItQuUpPfTQLxEPjteUzS7cKHuh30+ud3QHns/XxBvevOE+ZwJk9tIm2esGD/n3/cMkvZkbJjrKXH7NkmUyVfn3/cNU+zRfka0ne6EIkP3Ro0T2ff8Qeh3iV85+4BOOWMv2/x4q+75/dGnexjuXX+HLw9RlL2ysZd/3D+PDm3l7XSm/EHiNN2p84d/3j3OwjktvZvMUu7089YcT/Pv+oWYcywfOiucJbcl8bNJG/n3/0ObhfFhFKE/bGcQfFAagxxDfqjuPV3z14pUNQ3lOT3v0JOK1gb7ca6WMa1WZcXG6BD2N+ACdiXyttzWvjNHh2iOeYv5nE+920Z23t/blV9LfQqvJCfRtxD9Gc76nVocPMmmE4Ra56HuI+3dz4taLVHmaYwX82ykGvZS43NOGr5j8Esz3FsAv1j7olXT/bS14jc9NuL57PdT6CuiniefnG/EpqRcgUjcCTOsN0euIP0kx4OOLjkG/8BmQbaqJ3khcs02PR0wogUE1HqD/x3Psv7fo+aqQ8KV+BdDKHWHcmjr0B8TbrmtxIXcT5AabQKPHQfQW4omj1bnX42QwudMT5Jbf7gdviRtpduJRfWIhRlCHS/7x6P/S/e38Fe52Wwn1fdvZeptwdJWajj7YpB0q5i6CnY2vWLJPEHon4jD9HzDoHwx5Tg+Z7vZp6F2INx1tg7smc0Amvcn+eTYWXZ14uc1rcAv3h95/XmbevYaiaxIvrWqBBn1vOPL8HGv7UYquRXzv7Kew4ONESI2rZnV3B6NrE880ewQj+42BO1vKmcTbFF2XePjne3Amwh029DvAri7ti96duM+z27DxRzfgartYT5ue6D2IB7TcgtQXDPTHFrKF87uh6xE/pHoD7r51hj/fZLP3Q7qg9yIeNagJTlvbQ+6TDBZ1tR3vH32I68xthAPpNmA+OIUxo1foBsTzj1yGET9ZweLjiWyJy330fsQTe10Ej6dm8DFlDSvRbUY3JG4ZVwf5l4wheEckg5R6dCPiNzUugNff/eFC61IWVnYKfSBx7cxaWCcxgCVdQ9gfoUfRTYnbOp+BdbN6Qfd1c5jmkQPo5sSPPzwJB272AI8lfqx6VTG6JfEh+dXwe6gEhlVPYh41+ehWxLsHVcEYU21YGzCO/bggC92a+BNeCe/facK7qSPZmORUdBviw4yPQp97anBlmytL6JqMPoR4paQcVj/pBDulIht8bw36z8Qfdy0DFQ0VGKfvyDIkkegOxGdKSqFbRDtrcbRl+v0WozsRvzrgIOgYvGfNm35iqiXz0KXEbwv74YcHr9luB1PmVT8dXSA+aHoJQNNz5qw9gHmHTEZnxOdt2AOXWh+xVQYGzCV5LDonPur8LujvcI8d9NFnZwxHoMuJR2kVQ2r2X8z2cncmGnJ0N+JnvHbAPvPrLCJMm5VGO6EPI35p13bocrWRDXfVZC9G2qKPIO6nvg2khZeYt9CFXQyyRB9J3O63AlBNqWM7p6mw5tvG6KOIa13PA7Occ2zFiE9ielFfdE/iJZ5bYdnZ0yzNsE20rNBHH0v83vkcuKdVw/qzFtHth+7o44mHemaD52/H2fXsh+LHfVroE4mbNGfB0UcV7JBwW1yZrIb+K/HzczPBblkZq9a/LvoWqaBPIm6juhnOmx5icqsG0ffNB0GhmELcLD8D8p7sY1sX14uXVF+jTyUeNTwd7tTsYTPe14onlj5G9yHeszUVju0vZo47T4ot426j+xE/XLgR5pcWsSfxx8XimCb06cSlU1OA1xUyv8xysVb7EnoA8eW9N0D4P3ksr/GQmP3sHPos4hNvroMFDrmsTrpPXKV3En0O8ZzCZIiN3cJ61+4S3eOPoc8l7hCaBHqPN7OGZUVi8dDD6POIf3RLhKQpGSxgYoH4wHUf+u/EzxkkwJRbqYyPzRVTVxWjz6f79yEO2oJTWNtvWeKZj4XoocTVbqyFRsl6NmB3hjh3by56GHHn6jUQUZPEFmikivUZmeiLiLfvioXFqxPY7uj14s+H09CXEB+SFQMjPONYlkGSaNNlA3o48ZrkaIg0X8MMLsaJG6IT0ZcTXx8bBeU6McwgJ1a8YrMWfSXxyMhIiFaNYikxUeIXtWj0VcQlPBxGVYSy7J1BonR7AHoM8XzdeVD11Ys1NwwVi/Xt0ZOInw/0Be+VMtavykzM8JegpxE31pkIid7WrCZGR7R3f+qC/Y+49kV3+NLal7WkvxXOm51A30b3L5rDvlodxkwahR3mueh7iM/o5gQ2i1RZpmOFkNQ5Br2UuKunDaye/FL8eW+BoGXrg15JXM3WAk773BQf7F4v3PAV0E8TL8g3gmmpF8S1uhFCdb0heh3xpykGMLHomGgVPkPoba6J3ki8a5serJ5QIkprPATLP55Lsf8RN62QwDK/AvE9dxRU1tahPyD+9roWsNxNYlGwibBh9EH0FuJJo9Vh8uNkcdCdnsJoy2z0t8SNNTtBTJ9YMUFQF7b6xaP/S/yU5DPTsA0XA4PfucTvD0VXPdnRNyS8Y9GbgsQ0mzsufRy80NWIqy18yQZxH1H+a73L2XhA70q87NRjVttpvAjelS6Bne3QdYjXK/LkKv/HuDbgFsxKS2ZSab4w9m9N6ff+P1BLAQItAy0AAAAIAAAAIQBxd08zfAEAAJADAAAIAAAAAAAAAAAAAACAAQAAAABpZHhzLm5weVBLAQItAy0AAAAIAAAAIQCTRxsZDg0AAAAZAAAIAAAAAAAAAAAAAACAAbYBAAByb3dzLm5weVBLBQYAAAAAAgACAGwAAAD+DgAAAAA="

_PWP_PATCHED_DIR = None


def _ensure_act_tables():
    global _PWP_PATCHED_DIR
    if _PWP_PATCHED_DIR is not None:
        return
    import base64, io, shutil, tempfile
    from neuronxcc.driver.Job import Job
    from neuronxcc.driver.jobs.support.FindActInfo import findActInfoFile

    src_json = findActInfoFile(Job.getPackageDir(), "sunda")
    src_dir = os.path.dirname(src_json)
    d = tempfile.mkdtemp(prefix="pwp_g8_")
    for f in os.listdir(src_dir):
        shutil.copy(os.path.join(src_dir, f), os.path.join(d, f))
        os.chmod(os.path.join(d, f), 0o644)
    dat = np.load(io.BytesIO(base64.b64decode(_ACT_PATCH_B64)))
    bpath = os.path.join(d, "gelu_and_others_bkt.bin")
    b = np.fromfile(bpath, dtype=np.float32).reshape(-1, 8).copy()
    b[dat["idxs"]] = dat["rows"]
    b.tofile(bpath)
    os.environ["BASS_ACT_ROOT_JSON_PATH"] = os.path.join(d, "act_info.json")
    _PWP_PATCHED_DIR = d


def _register_consts(nc, vals):
    for v in vals:
        key = (F32, float(v))
        if key not in nc.const_aps.aps:
            t = nc.alloc_sbuf_tensor(f"const-f32-{float(v)}", [128, 1], F32)
            nc.gpsimd.memset(t.ap(), float(v))
            nc.const_aps.aps[key] = t.ap()


def build_graph(T=2048):
    _ensure_act_tables()
    nc = bacc.Bacc("TRN2", target_bir_lowering=False, debug=False)
    _register_consts(nc, [-1.0, -0.25, 0.5, 1.0 / math.pi, 1e-6])
    xT = nc.dram_tensor("xT", [D, T], F32, kind="ExternalInput").ap()
    wqT = nc.dram_tensor("wqT", [D, CL], F32, kind="ExternalInput").ap()
    wkT = nc.dram_tensor("wkT", [D, CL], F32, kind="ExternalInput").ap()
    wvT = nc.dram_tensor("wvT", [D, CL], F32, kind="ExternalInput").ap()
    woT = nc.dram_tensor("woT", [CL, D], F32, kind="ExternalInput").ap()
    cE = nc.dram_tensor("cE", [128, 2], F32, kind="ExternalInput").ap()
    cE2 = nc.dram_tensor("cE2", [2, 128], F32, kind="ExternalInput").ap()
    out = nc.dram_tensor("out", [T, D], F32, kind="ExternalOutput").ap()
    with tile.TileContext(nc) as tc:
        _body(tc, xT, wqT, wkT, wvT, woT, cE, cE2, out, T)
    nc.compile()
    return nc


def _body(tc, xT, wqT, wkT, wvT, woT, cE, cE2, out, T):
    nc = tc.nc
    ND = D // 128                 # 8 d-chunks
    NKB = T // 128                # key blocks
    TQ = min(512, T)              # moving-operand chunk
    NTC = T // TQ                 # t-chunks for projections
    WS = min(1024, T)             # elementwise supertile width
    NQCP = T // WS                # q superchunks
    NJ = WS // TQ                 # 512-chunks per supertile
    KB_PHASE = 8                  # k-blocks per ACT-table phase

    ach = _ActChain()
    with ExitStack() as top:
        tp = lambda **kw: top.enter_context(tc.tile_pool(**kw))

        const = tp(name="const", bufs=1)
        vonesf = const.tile([128, HL], F32)
        nc.gpsimd.memset(vonesf[:], 1.0)
        # E: [128, 2] head-halves indicator (lhsT for per-head sumsq);
        # E2 = E.T broadcasts [2,*] -> [128,*].  Host-supplied constants.
        E = const.tile([128, 2], F32R)
        nc.sync.dma_start(E[:], _r(cE[:]))
        E2 = const.tile([2, 128], F32R)
        nc.sync.dma_start(E2[:], _r(cE2[:]))

        # resident tensors
        wo_pool = tp(name="wo", bufs=1)
        wot = []
        for p_ in range(2):
            t_ = wo_pool.tile([128, D], F32R, name=f"wot{p_}", tag=f"wot{p_}")
            nc.sync.dma_start(t_[:], _r(woT[128 * p_:128 * (p_ + 1), :]))
            wot.append(t_)
        qk_res = tp(name="qk_res", bufs=1)
        qnT = [qk_res.tile([128, T], F32R, name=f"qnT{p}", tag=f"qnT{p}") for p in range(2)]
        knT = [qk_res.tile([128, T], F32R, name=f"knT{p}", tag=f"knT{p}") for p in range(2)]
        v_res = tp(name="v_res", bufs=1)
        vt = [v_res.tile([128, HL * 65], F32R, name=f"vt{i}", tag=f"vt{i}") for i in range(NKB)]
        on_res = tp(name="on_res", bufs=1)
        outnT = [on_res.tile([128, T], F32R, name=f"outnT{p}", tag=f"outnT{p}") for p in range(2)]

        # ---------------- phase 1: load + projections + norms ----------------
        with tc.tile_pool(name="xw", bufs=1) as xw, \
             tc.tile_pool(name="p1s", bufs=3) as p1s, \
             tc.tile_pool(name="p1ps", bufs=2, space="PSUM") as p1ps, \
             tc.tile_pool(name="p1ps2", bufs=2, space="PSUM") as p1ps2:
            xt = []
            for i in range(ND):
                t_ = xw.tile([128, T], F32R, name=f"xt{i}", tag=f"xt{i}")
                nc.sync.dma_start(t_[:], _r(xT[128 * i:128 * (i + 1), :]))
                xt.append(t_)
            wts = {}
            for nm, src in (("q", wqT), ("k", wkT), ("v", wvT)):
                wts[nm] = []
                for i in range(ND):
                    t_ = xw.tile([128, CL], F32R, name=f"w{nm}{i}", tag=f"w{nm}{i}")
                    nc.sync.dma_start(t_[:], _r(src[128 * i:128 * (i + 1), :]))
                    wts[nm].append(t_)

            def _proj_pair(wt, dstl, p):
                for tc_i in range(NTC):
                    P = p1ps.tile([128, TQ], F32, name="P", tag="projP")
                    for dc in range(ND):
                        nc.tensor.matmul(
                            P[:],
                            _r(wt[dc][:, 128 * p:128 * (p + 1)]),
                            _r(xt[dc][:, TQ * tc_i:TQ * (tc_i + 1)]),
                            start=(dc == 0), stop=(dc == ND - 1),
                        )
                    sq = p1s.tile([128, TQ], F32R, name="sq", tag="sq")
                    ach.add(nc.scalar.activation(sq[:], P[:], AF.Square))
                    ssq = p1ps2.tile([2, TQ], F32, name="ssq", tag="ssq")
                    nc.tensor.matmul(ssq[:], E[:], sq[:], start=True, stop=True)
                    inv = p1s.tile([2, TQ], F32R, name="inv", tag="inv")
                    ach.add(nc.scalar.activation(inv[:], ssq[:],
                                                 AF.Abs_reciprocal_sqrt))
                    bc = p1ps2.tile([128, TQ], F32, name="bc", tag="bc")
                    nc.tensor.matmul(bc[:], E2[:], inv[:], start=True, stop=True)
                    bcs = p1s.tile([128, TQ], F32, name="bcs", tag="bcs")
                    nc.vector.tensor_copy(bcs[:], bc[:])
                    nc.vector.tensor_mul(
                        dstl[p][:, TQ * tc_i:TQ * (tc_i + 1)], P[:], bcs[:])

            # pair 0 first so heads 0-1 (and V below) unblock the main loop early
            _proj_pair(wts["q"], qnT, 0)
            _proj_pair(wts["k"], knT, 0)

            # V (4 heads side by side, with a ones column after each head)
            for kb in range(NKB):
                P = p1ps.tile([128, CL], F32, tag="vP")
                for dc in range(ND):
                    nc.tensor.matmul(
                        P[:],
                        _r(xt[dc][:, 128 * kb:128 * (kb + 1)]),
                        _r(wts["v"][dc][:]),
                        start=(dc == 0), stop=(dc == ND - 1),
                    )
                vt_k = vt[kb]
                dst = vt_k[:].rearrange("p (h c) -> p h c", c=65)
                src = P[:].rearrange("p (h c) -> p h c", c=64)
                ach.add(nc.scalar.copy(dst[:, :, 0:64], src[:]))
                ach.add(nc.scalar.copy(
                    dst[:, :, 64:65],
                    vonesf[:].rearrange("p (a b) -> p a b", b=1)))

            _proj_pair(wts["q"], qnT, 1)
            _proj_pair(wts["k"], knT, 1)

        # ---------------- phase 2: attention main loop ----------------
        # Elementwise W = (0.5 + arcsin(s)/pi)^8 is ONE activation via a
        # custom piecewise-cubic table loaded into the gelu slot; the rowsum
        # reciprocal rides the derivative_gelu slot (same table set, so the
        # main loop never reloads ACT tables).
        p2 = ExitStack()
        tp2 = lambda **kw: p2.enter_context(tc.tile_pool(**kw))
        ps_s = tp2(name="ps_s", bufs=2, space="PSUM")    # S~ supertiles (2 banks ea)
        ps_gv = tp2(name="ps_gv", bufs=3, space="PSUM")
        ps_f = tp2(name="ps_f", bufs=1, space="PSUM")  # GV accumulators
        sc_g = tp2(name="sc_g", bufs=6)
        sc_n = tp2(name="sc_n", bufs=2)

        for qcp in range(NQCP):
            for h in range(HL):
                p, r0 = divmod(h, 2)
                qT = qnT[p][64 * r0:64 * r0 + 64, :]
                kT = knT[p][64 * r0:64 * r0 + 64, :]
                gv = [ps_gv.tile([65, TQ], F32, name="gv", tag="gv") for _ in range(NJ)]
                pend = []  # depth-2 software pipeline: AV(kb-2) after S~(kb)
                for kb in range(NKB):
                    st = ps_s.tile([128, WS], F32, tag="st")
                    for j in range(NJ):
                        nc.tensor.matmul(
                            st[:, TQ * j:TQ * (j + 1)],
                            kT[:, 128 * kb:128 * (kb + 1)],
                            qT[:, WS * qcp + TQ * j:WS * qcp + TQ * (j + 1)],
                            start=True, stop=True,
                        )
                    if len(pend) >= 2:
                        pkb, pg = pend.pop(0)
                        for j in range(NJ):
                            nc.tensor.matmul(
                                gv[j][:],
                                vt[pkb][:, 65 * h:65 * h + 65],
                                pg[:, TQ * j:TQ * (j + 1)],
                                start=(pkb == 0), stop=(pkb == NKB - 1),
                            )
                    g = sc_g.tile([128, WS], F32R, tag="g")
                    ach.add(nc.scalar.activation(g[:], st[:], AF.Gelu))
                    pend.append((kb, g))
                for pkb, pg in pend:
                    for j in range(NJ):
                        nc.tensor.matmul(
                            gv[j][:],
                            vt[pkb][:, 65 * h:65 * h + 65],
                            pg[:, TQ * j:TQ * (j + 1)],
                            start=(pkb == 0), stop=(pkb == NKB - 1),
                        )
                # normalize: out_h = GV[0:64] / (rowsum + 1e-6)
                # derivative_gelu slot: 1/(-4z + 1e-6); feed z = -rs/4.
                for j in range(NJ):
                    ninv = sc_n.tile([1, TQ], F32, name="ninv", tag="ninv")
                    # NOT in the ACT-order chain: Derivative_Gelu shares the
                    # gelu table set, and chaining it would stall the next
                    # head's GELUs behind this head's last AV matmul.
                    nc.scalar.activation(ninv[:], gv[j][64:65, :],
                                         AF.Derivative_Gelu, scale=-0.25)
                    invb = sc_n.tile([64, TQ], F32, name="ninvb", tag="ninvb")
                    nc.gpsimd.partition_broadcast(invb[:], ninv[:])
                    q_lo = WS * qcp + TQ * j
                    nc.vector.tensor_mul(
                        outnT[p][64 * r0:64 * r0 + 64, q_lo:q_lo + TQ],
                        gv[j][0:64, :], invb[:])
            # output projection for this qcp span (overlaps next qcp's work)
            for qb in range(WS * qcp // 128, WS * (qcp + 1) // 128):
                for oc in range(D // 512):
                    P = ps_f.tile([128, 512], F32, tag="fP")
                    for p_ in range(2):
                        nc.tensor.matmul(
                            P[:],
                            outnT[p_][:, 128 * qb:128 * (qb + 1)],
                            wot[p_][:, 512 * oc:512 * (oc + 1)],
                            start=(p_ == 0), stop=(p_ == 1),
                        )
                    stg = wo_pool.tile([128, 512], F32, tag="fstg", bufs=4)
                    nc.vector.tensor_copy(stg[:], P[:])
                    nc.sync.dma_start(
                        out[128 * qb:128 * (qb + 1), 512 * oc:512 * (oc + 1)],
                        stg[:])

        p2.close()


_NC_CACHE = {}


def _get_graph(T=2048):
    if T not in _NC_CACHE:
        _NC_CACHE[T] = build_graph(T)
    return _NC_CACHE[T]


def make_in_maps(x, Wq, Wk, Wv, Wo):
    x = np.asarray(x, dtype=np.float32)
    in_maps = []
    for c in range(N_CORES):
        b, g = divmod(c, 4)
        lo, hi = CL * g, CL * (g + 1)
        cE = np.zeros((128, 2), np.float32)
        cE[0:64, 0] = 1.0
        cE[64:128, 1] = 1.0
        in_maps.append({
            "cE": cE,
            "cE2": np.ascontiguousarray(cE.T),
            "xT": np.ascontiguousarray(x[b].T),
            "wqT": np.ascontiguousarray(np.asarray(Wq)[lo:hi, :].T),
            "wkT": np.ascontiguousarray(np.asarray(Wk)[lo:hi, :].T),
            "wvT": np.ascontiguousarray(np.asarray(Wv)[lo:hi, :].T),
            "woT": np.ascontiguousarray(np.asarray(Wo)[:, lo:hi].T),
        })
    return in_maps


def combine_outputs(results, bo, B=2, T=2048):
    out = np.zeros((B, T, D), dtype=np.float32)
    for c in range(N_CORES):
        out[c // 4] += results[c]["out"]
    out += np.asarray(bo, dtype=np.float32)[None, None, :]
    return out


def kernel(x, Wq, Wk, Wv, Wo, bo, _trace=False):
    _ensure_act_tables()
    from concourse.bass_utils import run_bass_kernel_spmd
    B, T, d = np.asarray(x).shape
    assert d == D
    nc = _get_graph(T)
    in_maps = make_in_maps(x, Wq, Wk, Wv, Wo)
    res = run_bass_kernel_spmd(nc, in_maps, core_ids=list(range(N_CORES)),
                               trace=_trace)
    out = combine_outputs(res.results, bo, B=B, T=T)
    if _trace:
        kernel.last_exec_time_ns = res.exec_time_ns
        kernel.last_results = res
    return out

